# revision 24
# baseline (speedup 1.0000x reference)
"""Trainium2 Bass kernel for nn_AudioGuidedVisualAttn (v2).

Math (per frame): 1-query cross-attention over N=196 visual patches + FFN +
residual + LayerNorm.  Exact algebraic reformulation (no approximation):

  scores[h,n] = visual[n] . u_h          u_h = Wk_h^T q_h_scaled  (const drops)
  attn_h      = Wv_h @ (sum_n w[h,n] visual[n]) + bv_h

v2 restructure vs v1:
  * visual cast fp32->fp16 on GpSimd; whole score/ctx path in fp16
  * quad (4-frame) processing: batched DMA, full-width softmax, batched
    z-normalization folded into the w-transpose (zdiag mover)
  * ctx via reversed matmul (w stationary M=32, visual moving, col-tiled
    4-concurrent) instead of per-frame visual weight reloads
  * CTXT built by a 16-column selector transpose per quad
  * FFN/projection GEMMs in fp16 (fast weight loads), LayerNorm in f32r

Sharding: pure data-parallel, batch B=16 split across 8 cores (2 batches =
120 frames per core), weights replicated.
"""

import os
import sys

sys.path.insert(0, "/opt/trn_rl_repo")

import numpy as np

import concourse.bass as bass
import concourse.mybir as mybir
import concourse.tile as tile
from concourse.masks import make_identity

F32 = mybir.dt.float32
F32R = mybir.dt.float32r
FP16 = mybir.dt.bfloat16 if os.environ.get("KHALF", "fp16") == "bf16" else mybir.dt.float16
I16 = mybir.dt.int16
AF = mybir.ActivationFunctionType
AX = mybir.AxisListType

B, T, N, C, H = 16, 60, 196, 512, 4
D = C // H  # 128
P = 128
CS = C // P  # 4 c-subtiles
NCH = [(0, 128), (128, 68)]  # n-dim chunks (offset, size)
EPS = 1e-5
NCORES = 8
FRAMES = (B // NCORES) * T  # 120 per core
NQ = FRAMES // 4  # 30 quads


def build_nc(F=FRAMES):
    KST = int(os.environ.get("KSTAGE", "99"))
    assert F % 4 == 0
    nq = F // 4
    nc = bass.Bass()

    audio = nc.dram_tensor("audio", [F, C], F32, kind="ExternalInput")
    visual = nc.dram_tensor("visual", [F, N, C], F32, kind="ExternalInput")
    # weights, host-prelaid as [128, CS, C] (partition, k-subtile, free)
    wqts = nc.dram_tensor("wqts", [P, CS, C], F32R, kind="ExternalInput")
    wkdhc = nc.dram_tensor("wkdhc", [P, H, C], F32R, kind="ExternalInput")
    wvt = nc.dram_tensor("wvt", [P, CS, C], FP16, kind="ExternalInput")
    woutt = nc.dram_tensor("woutt", [P, CS, C], FP16, kind="ExternalInput")
    lin1t = nc.dram_tensor("lin1t", [P, CS, C], FP16, kind="ExternalInput")
    lin2t = nc.dram_tensor("lin2t", [P, CS, C], FP16, kind="ExternalInput")
    biases = nc.dram_tensor("biases", [P, 6 * CS], F32, kind="ExternalInput")
    out = nc.dram_tensor("out", [F, C], F32, kind="ExternalOutput")

    with tile.TileContext(nc) as tc:
        with tc.tile_pool(name="const", bufs=1) as cpool, tc.tile_pool(
            name="persist", bufs=1
        ) as ppool:
            ident = cpool.tile([P, P], F32)
            make_identity(nc, ident)
            ident_h = cpool.tile([P, P], FP16, tag="identh")
            nc.scalar.activation(ident_h[:], ident[:], AF.Copy)
            ones = cpool.tile([P, 1], F32)
            nc.vector.memset(ones, 1.0)
            ones_r = cpool.tile([1, P], F32)
            nc.vector.memset(ones_r, 1.0)
            # ID4M[r, c] = 1 iff c == r%32 and r%32 < 4, fp16
            id32 = cpool.tile([P, 32], FP16, tag="id32")
            nc.vector.memset(id32[:].bitcast(I16), 0)
            for g in range(4):
                nc.vector.tensor_copy(
                    id32[32 * g : 32 * g + 4, 0:4],
                    ident_h[32 * g : 32 * g + 4, 32 * g : 32 * g + 4],
                )
            # SEL16[32*qi+h, 4*qi+h] = 1 (h<4), else 0, fp16
            sel16 = cpool.tile([P, 16], FP16, tag="sel16")
            nc.vector.memset(sel16[:].bitcast(I16), 0)
            for qi in range(4):
                nc.vector.tensor_copy(
                    sel16[32 * qi : 32 * qi + 4, 4 * qi : 4 * qi + 4],
                    ident_h[32 * qi : 32 * qi + 4, 32 * qi : 32 * qi + 4],
                )

            w_q = cpool.tile([P, CS, C], F32R, tag="wq")
            w_k = cpool.tile([P, H, C], F32R, tag="wk")
            w_v = cpool.tile([P, CS, C], FP16, tag="wv")
            w_o = cpool.tile([P, CS, C], FP16, tag="wo")
            w_1 = cpool.tile([P, CS, C], FP16, tag="w1")
            w_2 = cpool.tile([P, CS, C], FP16, tag="w2")
            bia = cpool.tile([P, 6 * CS], F32, tag="bias")
            nc.sync.dma_start(w_q[:], wqts[:])
            nc.sync.dma_start(w_k[:], wkdhc[:])
            nc.sync.dma_start(bia[:], biases[:])
            nc.sync.dma_start(w_v[:], wvt[:])
            nc.sync.dma_start(w_o[:], woutt[:])
            nc.sync.dma_start(w_1[:], lin1t[:])
            nc.sync.dma_start(w_2[:], lin2t[:])
            b_qs = bia[:, 0:CS]
            b_o = bia[:, CS : 2 * CS]
            b_1 = bia[:, 2 * CS : 3 * CS]
            b_2 = bia[:, 3 * CS : 4 * CS]
            ln_g = bia[:, 4 * CS : 5 * CS]
            lnb_t = bia[:, 5 * CS : 6 * CS]

            # U2[c%128, csub, pair*64 + 32*fi + h] = u[2*pair+fi, h, c], fp16
            # (scores lhsT: M=64 -> frame fi head-rows at psum partition 32*fi;
            # zero rows elsewhere keep the whole psum bank written/finite)
            U2 = ppool.tile([P, CS, (F // 2) * 64], FP16, tag="U2")
            nc.vector.memset(U2[:].bitcast(I16), 0)
            # CTXT[c%128, csub, 4*f + h] = sum_n w[f,h,n] visual[f,n,c] / Z[f,h]
            CTXT = ppool.tile([P, CS, 4 * F], FP16, tag="CTXT")
            if os.environ.get("KQUADS") or os.environ.get("KSTAGE"):
                nc.vector.memset(CTXT[:].bitcast(I16), 0)

            # ---------------- phase 1: qT and U ----------------
            with tc.tile_pool(name="ph1", bufs=2) as ph1, tc.tile_pool(
                name="ph1p", bufs=2, space="PSUM"
            ) as ph1p:
                audio_sb = ph1.tile([F, C], F32, tag="audio")
                nc.sync.dma_start(audio_sb[:], audio[:])
                audioT = ph1.tile([P, CS, F], F32R, tag="audioT")
                for k in range(CS):
                    pt = ph1p.tile([P, F], F32, tag="p1")
                    nc.tensor.transpose(
                        pt[:], audio_sb[:, k * P : (k + 1) * P], ident[0:F, 0:F]
                    )
                    nc.scalar.activation(audioT[:, k, :], pt[:], AF.Copy)
                qT = ph1.tile([P, H, F], F32R, tag="qT")
                for hc in range(H):
                    pq = ph1p.tile([P, F], F32, tag="p1")
                    for k in range(CS):
                        nc.tensor.matmul(
                            pq[:],
                            lhsT=w_q[:, k, hc * P : (hc + 1) * P],
                            rhs=audioT[:, k, :],
                            start=(k == 0),
                            stop=(k == CS - 1),
                        )
                    nc.scalar.activation(
                        qT[:, hc, :], pq[:], AF.Identity, bias=b_qs[:, hc : hc + 1]
                    )
                for h in range(H):
                    for k in range(CS):
                        pu = ph1p.tile([P, F], F32, tag="p1")
                        nc.tensor.matmul(
                            pu[:],
                            lhsT=w_k[:, h, k * P : (k + 1) * P],
                            rhs=qT[:, h, :],
                            start=True,
                            stop=True,
                        )
                        for fi in range(2):
                            srcap = pu[:, :].rearrange(
                                "p (pr b) -> p pr b", b=2
                            )[:, :, fi]
                            dstap = U2[:, k, :].rearrange(
                                "p (pr s) -> p pr s", s=64
                            )[:, :, 32 * fi + h]
                            nc.scalar.activation(dstap, srcap, AF.Copy)

            # ---------------- phase 2: quad loop ----------------
            def _bufs(name, dflt):
                return int(os.environ.get("KB_" + name, dflt))

            with tc.tile_pool(name="vf", bufs=_bufs("vf", 2)) as vfpool, tc.tile_pool(
                name="vb", bufs=_bufs("vb", 3)
            ) as vbpool, tc.tile_pool(name="vt", bufs=_bufs("vt", 2)) as vtpool, tc.tile_pool(
                name="wsb", bufs=_bufs("wsb", 2)
            ) as wpool, tc.tile_pool(name="sm", bufs=_bufs("sm", 2)) as smpool, tc.tile_pool(
                name="ctxe", bufs=_bufs("ctxe", 2)
            ) as cepool, tc.tile_pool(name="wts", bufs=_bufs("wts", 2)) as wtspool, tc.tile_pool(
                name="ptr", bufs=3, space="PSUM"
            ) as ptpool, tc.tile_pool(name="psc", bufs=1, space="PSUM"
            ) as pscpool, tc.tile_pool(name="pcx", bufs=1, space="PSUM"
            ) as pcxpool, tc.tile_pool(name="pwt", bufs=1, space="PSUM"
            ) as pwtpool, tc.tile_pool(name="pct", bufs=1, space="PSUM"
            ) as pctpool:
                for q in range(int(os.environ.get("KQUADS", nq) or nq)):
                    f0 = 4 * q
                    # ---- load + cast ----
                    vf = vfpool.tile([P, 4, 2, C], F32, tag="vf")
                    if os.environ.get("KDMA") == "frame":
                        for fi4 in range(4):
                            nc.sync.dma_start(
                                vf[:, fi4, 0, :], visual[f0 + fi4, 0:P, :]
                            )
                            nc.sync.dma_start(
                                vf[0:68, fi4, 1, :], visual[f0 + fi4, P:N, :]
                            )
                    else:
                        nc.sync.dma_start(
                            vf[:, :, 0, :],
                            visual[f0 : f0 + 4, 0:P, :].rearrange("f n c -> n f c"),
                        )
                        nc.sync.dma_start(
                            vf[0:68, :, 1, :],
                            visual[f0 : f0 + 4, P:N, :].rearrange("f n c -> n f c"),
                        )
                    vb = vbpool.tile([P, 4, 2, C], FP16, tag="vb")
                    if os.environ.get("KCAST") == "dve":
                        nc.vector.tensor_copy(vb[:, :, 0, :], vf[:, :, 0, :])
                        nc.vector.tensor_copy(vb[0:68, :, 1, :], vf[0:68, :, 1, :])
                    else:
                        nc.gpsimd.tensor_copy(vb[:, :, 0, :], vf[:, :, 0, :])
                        nc.gpsimd.tensor_copy(vb[0:68, :, 1, :], vf[0:68, :, 1, :])

                    if KST < 2:
                        continue
                    # ---- transpose visual (normal matmul, fp16) ----
                    # vt[c%128, k, pr, 196*fi + n] = visual[4q+2pr+fi, n, c]
                    vt = vtpool.tile([P, CS, 2, 392], FP16, tag="vt")
                    for pr in range(2):
                        for k in range(CS):
                            pt = ptpool.tile([P, 512], F32, tag="pt")
                            for fi in range(2):
                                qi = 2 * pr + fi
                                for j, (noff, nsz) in enumerate(NCH):
                                    nc.tensor.matmul(
                                        pt[:, 196 * fi + noff : 196 * fi + noff + nsz],
                                        lhsT=vb[0:nsz, qi, j, k * P : (k + 1) * P],
                                        rhs=ident_h[0:nsz, 0:nsz],
                                        start=(fi == 0 and j == 0),
                                        stop=(fi == 1 and j == 1),
                                        skip_group_check=True,
                                    )
                            dst = vt[:, k, pr, :]
                            if (pr + k) % 2 == 0:
                                nc.scalar.activation(dst, pt[:, 0:392], AF.Copy)
                            else:
                                nc.vector.tensor_copy(dst, pt[:, 0:392])

                    if KST < 3:
                        continue
                    # ---- scores (pair pr in its own psum bank, rows 0:64) ----
                    psc = pscpool.tile([64, 2, 512], F32, tag="psc")
                    for pr in range(2):
                        for k in range(CS):
                            nc.tensor.matmul(
                                psc[0:64, pr, 0:392],
                                lhsT=U2[:, k, 64 * (2 * q + pr) : 64 * (2 * q + pr) + 64],
                                rhs=vt[:, k, pr, :],
                                start=(k == 0),
                                stop=(k == CS - 1),
                            )

                    if KST < 4:
                        continue
                    # ---- softmax; valid quadrant (pr, fi): rows 32fi, cols 196fi
                    nmax = smpool.tile([64, 4], F32, tag="nmax")
                    wsb = wpool.tile([64, 2, 392], FP16, tag="wsb")
                    esum = smpool.tile([64, 4], F32, tag="esum")
                    for qi in range(4):
                        pr, side = qi // 2, qi % 2
                        cols = slice(196 * side, 196 * side + 196)
                        nc.vector.reduce_max(
                            nmax[:, qi : qi + 1], psc[0:64, pr, cols],
                            axis=AX.X, negate=True,
                        )
                        nc.scalar.activation(
                            wsb[:, pr, cols], psc[0:64, pr, cols], AF.Exp,
                            bias=nmax[:, qi : qi + 1],
                            accum_out=esum[:, qi : qi + 1],
                        )
                    zinv = smpool.tile([64, 4], F32, tag="zinv")
                    nc.vector.reciprocal(zinv[:], esum[:])
                    # zdg[32fi+h, qi, c] = zinv[qi] * (c == h < 4), fp16
                    zdg = smpool.tile([64, 4, 32], FP16, tag="zdg")
                    for qi in range(4):
                        nc.vector.tensor_scalar_mul(
                            zdg[:, qi, :], id32[0:64, :], zinv[:, qi : qi + 1]
                        )

                    if KST < 5:
                        continue
                    # ---- w transpose: pwt[n, j, 32qi+c] = w~[qi, c, n] ----
                    pwt = pwtpool.tile([P, 2, P], F32, tag="pwt")
                    for qi in range(4):
                        pr, side = qi // 2, qi % 2
                        rows = slice(32 * side, 32 * side + 32)
                        for j, (noff, nsz) in enumerate(NCH):
                            nc.tensor.matmul(
                                pwt[0:nsz, j, 32 * qi : 32 * qi + 32],
                                lhsT=wsb[rows, pr, 196 * side + noff : 196 * side + noff + nsz],
                                rhs=(ident_h[rows, 0:32] if os.environ.get("KWRHS") == "ident" else zdg[rows, qi, :]),
                                start=True,
                                stop=True,
                                tile_position=(32 * side, 0),
                            )
                    wt_sb = wtspool.tile([P, 2, P], FP16, tag="wt")
                    nc.scalar.activation(wt_sb[:, 0, :], pwt[:, 0, :], AF.Copy)
                    nc.scalar.activation(wt_sb[0:68, 1, :], pwt[0:68, 1, :], AF.Copy)

                    if KST < 6:
                        continue
                    # ---- ctx: pcx[32qi+h, c] = sum_n w~ * visual ----
                    pcx = pcxpool.tile([P, C], F32, tag="pcx")
                    for qi in range(4):
                        for j, (noff, nsz) in enumerate(NCH):
                            nc.tensor.matmul(
                                pcx[32 * qi : 32 * qi + 32, :],
                                lhsT=wt_sb[0:nsz, j, 32 * qi : 32 * qi + 32],
                                rhs=vb[0:nsz, qi, j, :],
                                start=(j == 0),
                                stop=(j == 1),
                                tile_position=(0, 32 * qi),
                            )
                    ctxe = cepool.tile([P, C], FP16, tag="ctxe")
                    nc.vector.tensor_copy(ctxe[:], pcx[:])

                    if KST < 7:
                        continue
                    # ---- selector transpose -> CTXT[c, csub, 4f+h] ----
                    pct = pctpool.tile([P, CS, 16], F32, tag="pct")
                    for k in range(CS):
                        nc.tensor.matmul(
                            pct[:, k, :],
                            lhsT=ctxe[:, k * P : (k + 1) * P],
                            rhs=sel16[:],
                            start=(k == 0),
                            stop=(k == CS - 1),
                            skip_group_check=True,
                        )
                    nc.vector.tensor_copy(CTXT[:, :, 16 * q : 16 * q + 16], pct[:])

            # ---------------- phase 3: tail ----------------
            with tc.tile_pool(name="ph3", bufs=2) as ph3, tc.tile_pool(
                name="ph3p", bufs=2, space="PSUM"
            ) as ph3p, tc.tile_pool(
                name="ph3pn", bufs=1, space="PSUM"
            ) as ph3pn, tc.tile_pool(name="ph3po", bufs=1, space="PSUM") as ph3po:
                ap_sb = ph3.tile([P, H, F], FP16, tag="apre")
                for h in range(H):
                    pa = ph3p.tile([P, F], F32, tag="pt3")
                    ctxh = CTXT.rearrange("p s (f h) -> p s f h", h=H)
                    for k in range(CS):
                        nc.tensor.matmul(
                            pa[:],
                            lhsT=w_v[:, k, h * P : (h + 1) * P],
                            rhs=ctxh[:, k, :, h],
                            start=(k == 0),
                            stop=(k == CS - 1),
                        )
                    nc.scalar.activation(ap_sb[:, h, :], pa[:], AF.Copy)
                attnT = ph3.tile([P, CS, F], F32, tag="attnT")
                attnT16 = ph3.tile([P, CS, F], FP16, tag="attnT16")
                for ic in range(CS):
                    pb = ph3p.tile([P, F], F32, tag="pt3")
                    for s in range(CS):
                        nc.tensor.matmul(
                            pb[:],
                            lhsT=w_o[:, s, ic * P : (ic + 1) * P],
                            rhs=ap_sb[:, s, :],
                            start=(s == 0),
                            stop=(s == CS - 1),
                        )
                    nc.scalar.activation(
                        attnT[:, ic, :], pb[:], AF.Identity, bias=b_o[:, ic : ic + 1]
                    )
                    nc.vector.tensor_copy(attnT16[:, ic, :], attnT[:, ic, :])
                h1T = ph3.tile([P, CS, F], FP16, tag="h1T")
                for ic in range(CS):
                    pc = ph3p.tile([P, F], F32, tag="pt3")
                    for s in range(CS):
                        nc.tensor.matmul(
                            pc[:],
                            lhsT=w_1[:, s, ic * P : (ic + 1) * P],
                            rhs=attnT16[:, s, :],
                            start=(s == 0),
                            stop=(s == CS - 1),
                        )
                    nc.scalar.activation(
                        h1T[:, ic, :], pc[:], AF.Relu, bias=b_1[:, ic : ic + 1]
                    )
                xT = ph3.tile([P, CS, F], F32, tag="xT")
                for ic in range(CS):
                    pd = ph3p.tile([P, F], F32, tag="pt3")
                    for s in range(CS):
                        nc.tensor.matmul(
                            pd[:],
                            lhsT=w_2[:, s, ic * P : (ic + 1) * P],
                            rhs=h1T[:, s, :],
                            start=(s == 0),
                            stop=(s == CS - 1),
                        )
                    srcb = ph3.tile([P, F], F32, tag="srcb")
                    nc.scalar.activation(
                        srcb[:], pd[:], AF.Identity, bias=b_2[:, ic : ic + 1]
                    )
                    nc.vector.tensor_add(
                        out=xT[:, ic, :], in0=srcb[:], in1=attnT[:, ic, :]
                    )
                # LayerNorm over c (partition+subtile dim) via ones-matmul
                x2 = ph3.tile([P, CS, F], F32, tag="x2")
                nc.vector.tensor_mul(out=x2[:], in0=xT[:], in1=xT[:])
                psums = ph3pn.tile([1, 2, F], F32, tag="psums")
                ps1 = psums[:, 0, :]
                ps2 = psums[:, 1, :]
                for k in range(CS):
                    nc.tensor.matmul(
                        ps1, lhsT=ones[:, 0:1], rhs=xT[:, k, :],
                        start=(k == 0), stop=(k == CS - 1),
                    )
                for k in range(CS):
                    nc.tensor.matmul(
                        ps2, lhsT=ones[:, 0:1], rhs=x2[:, k, :],
                        start=(k == 0), stop=(k == CS - 1),
                    )
                mu = ph3.tile([1, F], F32, tag="mu")
                nc.scalar.activation(mu[:], ps1, AF.Copy, scale=1.0 / C)
                ms = ph3.tile([1, F], F32, tag="ms")
                nc.scalar.activation(ms[:], ps2, AF.Copy, scale=1.0 / C)
                mu2 = ph3.tile([1, F], F32, tag="mu2")
                nc.vector.tensor_mul(out=mu2[:], in0=mu[:], in1=mu[:])
                var = ph3.tile([1, F], F32, tag="var")
                nc.vector.tensor_tensor(
                    var[:], ms[:], mu2[:], mybir.AluOpType.subtract
                )
                epst = ph3.tile([1, 1], F32, tag="epst")
                nc.vector.memset(epst[:], EPS)
                std = ph3.tile([1, F], F32, tag="std")
                nc.scalar.activation(std[:], var[:], AF.Sqrt, bias=epst[0:1, 0:1])
                rstd = ph3.tile([1, F], F32, tag="rstd")
                nc.vector.reciprocal(rstd[:], std[:])
                mrs = ph3.tile([1, F], F32, tag="mrs")
                nc.vector.tensor_mul(out=mrs[:], in0=mu[:], in1=rstd[:])
                rstd_bc = ph3pn.tile([P, F], F32, tag="pbn1")
                nc.tensor.matmul(
                    rstd_bc[:], lhsT=ones_r[0:1, :], rhs=rstd[:], start=True, stop=True
                )
                mrs_bc = ph3pn.tile([P, F], F32, tag="pbn2")
                nc.tensor.matmul(
                    mrs_bc[:], lhsT=ones_r[0:1, :], rhs=mrs[:], start=True, stop=True
                )
                xn = ph3.tile([P, CS, F], F32, tag="xn")
                for k in range(CS):
                    tta = ph3.tile([P, F], F32, tag="tta")
                    nc.vector.tensor_mul(
                        out=tta[:], in0=xT[:, k, :], in1=rstd_bc[:]
                    )
                    ttb = ph3.tile([P, F], F32, tag="ttb")
                    nc.vector.tensor_tensor(
                        ttb[:], tta[:], mrs_bc[:], mybir.AluOpType.subtract,
                    )
                    nc.vector.tensor_scalar(
                        xn[:, k, :], ttb[:],
                        ln_g[:, k : k + 1], lnb_t[:, k : k + 1],
                        mybir.AluOpType.mult, mybir.AluOpType.add,
                    )
                # transpose back to [f, c] and store
                pout = ph3po.tile([F, C], F32, tag="pout")
                for k in range(CS):
                    nc.tensor.transpose(
                        pout[:, k * P : (k + 1) * P], xn[:, k, :], ident[0:P, 0:P]
                    )
                out_sb = ph3.tile([F, C], F32, tag="outsb")
                nc.scalar.activation(out_sb[:], pout[:], AF.Copy)
                nc.sync.dma_start(out[:], out_sb[:])
    _split_multi_waits(nc)
    return nc


def _split_multi_waits(nc):
    """This walrus build allows only one sync-wait per instruction struct;
    move extra waits onto single-wait NoOps on the same engine, inserted
    immediately before the instruction (same-engine program order makes
    this equivalent)."""
    import bass_rust

    n = [0]
    for func in nc.m.functions:
        for blk in func.blocks:
            insts = blk.instructions
            out = []
            for inst in insts:
                si = inst.sync_info
                waits = list(si.on_wait) if si and si.on_wait else []
                if len(waits) > 1 and inst.engine != mybir.EngineType.Unassigned:
                    for w in waits[:-1]:
                        nop = mybir.InstNoOp(
                            name=f"I-waitsplit-{n[0]}", ins=[], outs=[]
                        )
                        n[0] += 1
                        nop.engine = inst.engine
                        nop.sync_info = bass_rust.SyncInfo(
                            on_wait=[w], on_update=[]
                        )
                        nc.register_instruction(nop)
                        out.append(nop)
                    si.on_wait = [waits[-1]]
                out.append(inst)
            if len(out) != len(insts):
                insts.clear()
                insts.extend(out)
    return nc


def _patch_tile_drain():
    """This walrus build rejects >1 sync-wait on CTRL-class (Drain) instrs;
    split the Tile kernel-tail drain's waits into a chain of 1-wait drains."""
    import bass_rust
    from concourse.tile import ScopedClock

    if getattr(tile.TileContext, "_drain_patched", False):
        return

    def patched(self, tick_clock, wait_clock):
        drain_inst = self.nc.sync.drain()
        wait_clock.add_sem_waits(
            drain_inst.ins, ScopedClock({None: tick_clock.global_clock})
        )
        si = drain_inst.ins.sync_info
        waits = list(si.on_wait) if si and si.on_wait else []
        if len(waits) > 1:
            si.on_wait = [waits[0]]
            for w in waits[1:]:
                d2 = self.nc.sync.drain()
                d2.ins.sync_info = bass_rust.SyncInfo(on_wait=[w], on_update=[])
        self.nc.all_engine_barrier()
        popped = self.nc._tile_sem_poison_stack.pop()
        assert popped is self._sem_poison
        self.nc.clear_and_free_semaphores(list(self.sems.allocated().values()))
        self.nc.all_engine_barrier()

    tile.TileContext._drain_and_barrier = patched
    tile.TileContext._drain_patched = True


_patch_tile_drain()


def host_weights(in_proj_w, in_proj_b, out_proj_w, out_proj_b, lin1_w, lin1_b,
                 lin2_w, lin2_b, ln_g, ln_b):
    """Pre-lay weights into the [128, sub, free] SBUF layouts the kernel uses."""
    scale = 1.0 / np.sqrt(np.float32(D))
    Wq, Wk, Wv = (np.asarray(in_proj_w[i * C : (i + 1) * C]) for i in range(3))
    bq = np.asarray(in_proj_b[0:C])
    bv = np.asarray(in_proj_b[2 * C : 3 * C])

    def t_psf(w):  # [C_out rows, x] -> [p, sub, x] with rows = sub*128+p
        return np.ascontiguousarray(w.reshape(CS, P, -1).transpose(1, 0, 2))

    wqts = t_psf(np.ascontiguousarray(Wq.T) * scale)      # [c] rows -> q cols
    wkdhc = t_psf(Wk)                                     # [(h d), c] -> [d, h, c]
    wvt = t_psf(np.ascontiguousarray(Wv.T))               # [c, (h d)]
    woutt = t_psf(np.ascontiguousarray(np.asarray(out_proj_w).T))
    lin1t = t_psf(np.ascontiguousarray(np.asarray(lin1_w).T))
    lin2t = t_psf(np.ascontiguousarray(np.asarray(lin2_w).T))

    def b_ps(b):  # [512] -> [128, 4] with c = sub*128+p
        return np.ascontiguousarray(np.asarray(b).reshape(CS, P).T)

    b_o_eff = np.asarray(out_proj_b) + np.asarray(out_proj_w) @ bv
    biases = np.concatenate(
        [b_ps(bq * scale), b_ps(b_o_eff), b_ps(np.asarray(lin1_b)),
         b_ps(np.asarray(lin2_b)), b_ps(np.asarray(ln_g)),
         b_ps(np.asarray(ln_b))], axis=1,
    ).astype(np.float32)
    import ml_dtypes
    hdt = (ml_dtypes.bfloat16 if os.environ.get("KHALF", "fp16") == "bf16"
           else np.float16)
    return dict(
        wqts=wqts.astype(np.float32), wkdhc=wkdhc.astype(np.float32),
        wvt=wvt.astype(hdt), woutt=woutt.astype(hdt),
        lin1t=lin1t.astype(hdt), lin2t=lin2t.astype(hdt),
        biases=biases,
    )


_NC_CACHE = {}


def kernel(audio_top_k, visual_patch_feat, in_proj_w, in_proj_b, out_proj_w,
           out_proj_b, lin1_w, lin1_b, lin2_w, lin2_b, ln_g, ln_b):
    from concourse.bass_utils import run_bass_kernel_spmd

    wmap = host_weights(in_proj_w, in_proj_b, out_proj_w, out_proj_b,
                        lin1_w, lin1_b, lin2_w, lin2_b, ln_g, ln_b)
    audio = np.asarray(audio_top_k, np.float32)
    visual = np.asarray(visual_patch_feat, np.float32)
    bpc = B // NCORES
    in_maps = []
    for c in range(NCORES):
        sl = slice(c * bpc, (c + 1) * bpc)
        in_maps.append(
            dict(
                audio=np.ascontiguousarray(audio[sl].reshape(FRAMES, C)),
                visual=np.ascontiguousarray(visual[sl].reshape(FRAMES, N, C)),
                **wmap,
            )
        )
    if "nc" not in _NC_CACHE:
        _NC_CACHE["nc"] = build_nc()
    res = run_bass_kernel_spmd(_NC_CACHE["nc"], in_maps, list(range(NCORES)))
    outs = [res.results[c]["out"].reshape(bpc, T, C) for c in range(NCORES)]
    return np.concatenate(outs, axis=0)


# revision 26
# speedup vs baseline: 1.1825x; 1.1825x over previous
"""Trainium2 Bass kernel for nn_AudioGuidedVisualAttn (v2).

Math (per frame): 1-query cross-attention over N=196 visual patches + FFN +
residual + LayerNorm.  Exact algebraic reformulation (no approximation):

  scores[h,n] = visual[n] . u_h          u_h = Wk_h^T q_h_scaled  (const drops)
  attn_h      = Wv_h @ (sum_n w[h,n] visual[n]) + bv_h

v2 restructure vs v1:
  * visual cast fp32->fp16 on GpSimd; whole score/ctx path in fp16
  * quad (4-frame) processing: batched DMA, full-width softmax, batched
    z-normalization folded into the w-transpose (zdiag mover)
  * ctx via reversed matmul (w stationary M=32, visual moving, col-tiled
    4-concurrent) instead of per-frame visual weight reloads
  * CTXT built by a 16-column selector transpose per quad
  * FFN/projection GEMMs in fp16 (fast weight loads), LayerNorm in f32r

Sharding: pure data-parallel, batch B=16 split across 8 cores (2 batches =
120 frames per core), weights replicated.
"""

import os
import sys

sys.path.insert(0, "/opt/trn_rl_repo")

import numpy as np

import concourse.bass as bass
import concourse.mybir as mybir
import concourse.tile as tile
from concourse.masks import make_identity

F32 = mybir.dt.float32
F32R = mybir.dt.float32r
FP16 = mybir.dt.bfloat16 if os.environ.get("KHALF", "fp16") == "bf16" else mybir.dt.float16
I16 = mybir.dt.int16
AF = mybir.ActivationFunctionType
AX = mybir.AxisListType

B, T, N, C, H = 16, 60, 196, 512, 4
D = C // H  # 128
P = 128
CS = C // P  # 4 c-subtiles
NCH = [(0, 128), (128, 68)]  # n-dim chunks (offset, size)
EPS = 1e-5
NCORES = 8
FRAMES = (B // NCORES) * T  # 120 per core
NQ = FRAMES // 4  # 30 quads


def build_nc(F=FRAMES):
    KST = int(os.environ.get("KSTAGE", "99"))
    assert F % 4 == 0
    nq = F // 4
    nc = bass.Bass()

    audio = nc.dram_tensor("audio", [F, C], F32, kind="ExternalInput")
    visual = nc.dram_tensor("visual", [F, N, C], F32, kind="ExternalInput")
    # weights, host-prelaid as [128, CS, C] (partition, k-subtile, free)
    wqts = nc.dram_tensor("wqts", [P, CS, C], F32R, kind="ExternalInput")
    wkdhc = nc.dram_tensor("wkdhc", [P, H, C], F32R, kind="ExternalInput")
    wvt = nc.dram_tensor("wvt", [P, CS, C], FP16, kind="ExternalInput")
    woutt = nc.dram_tensor("woutt", [P, CS, C], FP16, kind="ExternalInput")
    lin1t = nc.dram_tensor("lin1t", [P, CS, C], FP16, kind="ExternalInput")
    lin2t = nc.dram_tensor("lin2t", [P, CS, C], FP16, kind="ExternalInput")
    biases = nc.dram_tensor("biases", [P, 6 * CS], F32, kind="ExternalInput")
    out = nc.dram_tensor("out", [F, C], F32, kind="ExternalOutput")

    with tile.TileContext(nc) as tc:
        with tc.tile_pool(name="const", bufs=1) as cpool, tc.tile_pool(
            name="persist", bufs=1
        ) as ppool:
            ident = cpool.tile([P, P], F32)
            make_identity(nc, ident)
            ident_h = cpool.tile([P, P], FP16, tag="identh")
            nc.scalar.activation(ident_h[:], ident[:], AF.Copy)
            ones = cpool.tile([P, 1], F32)
            nc.vector.memset(ones, 1.0)
            ones_r = cpool.tile([1, P], F32)
            nc.vector.memset(ones_r, 1.0)
            # ID4M[r, c] = 1 iff c == r%32 and r%32 < 4, fp16
            id32 = cpool.tile([P, 32], FP16, tag="id32")
            nc.vector.memset(id32[:].bitcast(I16), 0)
            for g in range(4):
                nc.vector.tensor_copy(
                    id32[32 * g : 32 * g + 4, 0:4],
                    ident_h[32 * g : 32 * g + 4, 32 * g : 32 * g + 4],
                )
            # SEL16[32*qi+h, 4*qi+h] = 1 (h<4), else 0, fp16
            sel16 = cpool.tile([P, 16], FP16, tag="sel16")
            nc.vector.memset(sel16[:].bitcast(I16), 0)
            for qi in range(4):
                nc.vector.tensor_copy(
                    sel16[32 * qi : 32 * qi + 4, 4 * qi : 4 * qi + 4],
                    ident_h[32 * qi : 32 * qi + 4, 32 * qi : 32 * qi + 4],
                )

            w_q = cpool.tile([P, CS, C], F32R, tag="wq")
            w_k = cpool.tile([P, H, C], F32R, tag="wk")
            w_v = cpool.tile([P, CS, C], FP16, tag="wv")
            w_o = cpool.tile([P, CS, C], FP16, tag="wo")
            w_1 = cpool.tile([P, CS, C], FP16, tag="w1")
            w_2 = cpool.tile([P, CS, C], FP16, tag="w2")
            bia = cpool.tile([P, 6 * CS], F32, tag="bias")
            nc.sync.dma_start(w_q[:], wqts[:])
            nc.sync.dma_start(w_k[:], wkdhc[:])
            nc.sync.dma_start(bia[:], biases[:])
            nc.sync.dma_start(w_v[:], wvt[:])
            nc.sync.dma_start(w_o[:], woutt[:])
            nc.sync.dma_start(w_1[:], lin1t[:])
            nc.sync.dma_start(w_2[:], lin2t[:])
            b_qs = bia[:, 0:CS]
            b_o = bia[:, CS : 2 * CS]
            b_1 = bia[:, 2 * CS : 3 * CS]
            b_2 = bia[:, 3 * CS : 4 * CS]
            ln_g = bia[:, 4 * CS : 5 * CS]
            lnb_t = bia[:, 5 * CS : 6 * CS]

            # U2[c%128, csub, pair*64 + 32*fi + h] = u[2*pair+fi, h, c], fp16
            # (scores lhsT: M=64 -> frame fi head-rows at psum partition 32*fi;
            # zero rows elsewhere keep the whole psum bank written/finite)
            U2 = ppool.tile([P, CS, (F // 2) * 64], FP16, tag="U2")
            nc.vector.memset(U2[:].bitcast(I16), 0)
            # CTXT[c%128, csub, 4*f + h] = sum_n w[f,h,n] visual[f,n,c] / Z[f,h]
            CTXT = ppool.tile([P, CS, 4 * F], FP16, tag="CTXT")
            if os.environ.get("KQUADS") or os.environ.get("KSTAGE"):
                nc.vector.memset(CTXT[:].bitcast(I16), 0)

            # ---------------- phase 1: qT and U ----------------
            with tc.tile_pool(name="ph1", bufs=2) as ph1, tc.tile_pool(
                name="ph1p", bufs=2, space="PSUM"
            ) as ph1p:
                audio_sb = ph1.tile([F, C], F32, tag="audio")
                nc.sync.dma_start(audio_sb[:], audio[:])
                audioT = ph1.tile([P, CS, F], F32R, tag="audioT")
                for k in range(CS):
                    pt = ph1p.tile([P, F], F32, tag="p1")
                    nc.tensor.transpose(
                        pt[:], audio_sb[:, k * P : (k + 1) * P], ident[0:F, 0:F]
                    )
                    nc.scalar.activation(audioT[:, k, :], pt[:], AF.Copy)
                qT = ph1.tile([P, H, F], F32R, tag="qT")
                for hc in range(H):
                    pq = ph1p.tile([P, F], F32, tag="p1")
                    for k in range(CS):
                        nc.tensor.matmul(
                            pq[:],
                            lhsT=w_q[:, k, hc * P : (hc + 1) * P],
                            rhs=audioT[:, k, :],
                            start=(k == 0),
                            stop=(k == CS - 1),
                        )
                    nc.scalar.activation(
                        qT[:, hc, :], pq[:], AF.Identity, bias=b_qs[:, hc : hc + 1]
                    )
                for h in range(H):
                    for k in range(CS):
                        pu = ph1p.tile([P, F], F32, tag="p1")
                        nc.tensor.matmul(
                            pu[:],
                            lhsT=w_k[:, h, k * P : (k + 1) * P],
                            rhs=qT[:, h, :],
                            start=True,
                            stop=True,
                        )
                        for fi in range(2):
                            srcap = pu[:, :].rearrange(
                                "p (pr b) -> p pr b", b=2
                            )[:, :, fi]
                            dstap = U2[:, k, :].rearrange(
                                "p (pr s) -> p pr s", s=64
                            )[:, :, 32 * fi + h]
                            nc.scalar.activation(dstap, srcap, AF.Copy)

            # ---------------- phase 2: quad loop ----------------
            def _bufs(name, dflt):
                return int(os.environ.get("KB_" + name, dflt))

            with tc.tile_pool(name="vf", bufs=_bufs("vf", 2)) as vfpool, tc.tile_pool(
                name="vb", bufs=_bufs("vb", 3)
            ) as vbpool, tc.tile_pool(name="vt", bufs=_bufs("vt", 2)) as vtpool, tc.tile_pool(
                name="wsb", bufs=_bufs("wsb", 2)
            ) as wpool, tc.tile_pool(name="sm", bufs=_bufs("sm", 2)) as smpool, tc.tile_pool(
                name="ctxe", bufs=_bufs("ctxe", 2)
            ) as cepool, tc.tile_pool(name="wts", bufs=_bufs("wts", 2)) as wtspool, tc.tile_pool(
                name="ptr", bufs=3, space="PSUM"
            ) as ptpool, tc.tile_pool(name="psc", bufs=1, space="PSUM"
            ) as pscpool, tc.tile_pool(name="pcx", bufs=1, space="PSUM"
            ) as pcxpool, tc.tile_pool(name="pwt", bufs=1, space="PSUM"
            ) as pwtpool, tc.tile_pool(name="pct", bufs=1, space="PSUM"
            ) as pctpool:
                for q in range(int(os.environ.get("KQUADS", nq) or nq)):
                    f0 = 4 * q
                    # ---- load + cast ----
                    vf = vfpool.tile([P, 4, 2, C], F32, tag="vf")
                    if os.environ.get("KDMA") == "frame":
                        for fi4 in range(4):
                            nc.sync.dma_start(
                                vf[:, fi4, 0, :], visual[f0 + fi4, 0:P, :]
                            )
                            nc.sync.dma_start(
                                vf[0:68, fi4, 1, :], visual[f0 + fi4, P:N, :]
                            )
                    else:
                        nc.sync.dma_start(
                            vf[:, :, 0, :],
                            visual[f0 : f0 + 4, 0:P, :].rearrange("f n c -> n f c"),
                        )
                        nc.sync.dma_start(
                            vf[0:68, :, 1, :],
                            visual[f0 : f0 + 4, P:N, :].rearrange("f n c -> n f c"),
                        )
                    vb = vbpool.tile([P, 4, 2, C], FP16, tag="vb")
                    if os.environ.get("KCAST") == "dve":
                        nc.vector.tensor_copy(vb[:, :, 0, :], vf[:, :, 0, :])
                        nc.vector.tensor_copy(vb[0:68, :, 1, :], vf[0:68, :, 1, :])
                    else:
                        nc.gpsimd.tensor_copy(vb[:, :, 0, :], vf[:, :, 0, :])
                        nc.gpsimd.tensor_copy(vb[0:68, :, 1, :], vf[0:68, :, 1, :])

                    if KST < 2:
                        continue
                    # ---- transpose visual (normal matmul, fp16) ----
                    # vt[c%128, k, pr, 196*fi + n] = visual[4q+2pr+fi, n, c]
                    vt = vtpool.tile([P, CS, 2, 392], FP16, tag="vt")
                    for pr in range(2):
                        for k in range(CS):
                            pt = ptpool.tile([P, 512], F32, tag="pt")
                            for fi in range(2):
                                qi = 2 * pr + fi
                                for j, (noff, nsz) in enumerate(NCH):
                                    nc.tensor.matmul(
                                        pt[:, 196 * fi + noff : 196 * fi + noff + nsz],
                                        lhsT=vb[0:nsz, qi, j, k * P : (k + 1) * P],
                                        rhs=ident_h[0:nsz, 0:nsz],
                                        start=(fi == 0 and j == 0),
                                        stop=(fi == 1 and j == 1),
                                        skip_group_check=True,
                                    )
                            dst = vt[:, k, pr, :]
                            if (pr + k) % 2 == 0:
                                nc.scalar.activation(dst, pt[:, 0:392], AF.Copy)
                            else:
                                nc.vector.tensor_copy(dst, pt[:, 0:392])

                    if KST < 3:
                        continue
                    # ---- scores (pair pr in its own psum bank, rows 0:64) ----
                    psc = pscpool.tile([64, 2, 512], F32, tag="psc")
                    for pr in range(2):
                        for k in range(CS):
                            nc.tensor.matmul(
                                psc[0:64, pr, 0:392],
                                lhsT=U2[:, k, 64 * (2 * q + pr) : 64 * (2 * q + pr) + 64],
                                rhs=vt[:, k, pr, :],
                                start=(k == 0),
                                stop=(k == CS - 1),
                            )

                    if KST < 4:
                        continue
                    # ---- softmax; valid quadrant (pr, fi): rows 32fi, cols 196fi
                    nmax = smpool.tile([64, 4], F32, tag="nmax")
                    wsb = wpool.tile([64, 2, 392], FP16, tag="wsb")
                    esum = smpool.tile([64, 4], F32, tag="esum")
                    for qi in range(4):
                        pr, side = qi // 2, qi % 2
                        cols = slice(196 * side, 196 * side + 196)
                        nc.vector.reduce_max(
                            nmax[:, qi : qi + 1], psc[0:64, pr, cols],
                            axis=AX.X, negate=True,
                        )
                        nc.scalar.activation(
                            wsb[:, pr, cols], psc[0:64, pr, cols], AF.Exp,
                            bias=nmax[:, qi : qi + 1],
                            accum_out=esum[:, qi : qi + 1],
                        )
                    zinv = smpool.tile([64, 4], F32, tag="zinv")
                    if os.environ.get("KWZ") == "norecip":
                        nc.vector.tensor_copy(zinv[:], esum[:])
                    else:
                        nc.vector.reciprocal(zinv[:], esum[:])
                    # zdg[32fi+h, qi, c] = zinv[qi] * (c == h < 4), fp16
                    zdg = smpool.tile([64, 4, 32], FP16, tag="zdg")
                    for qi in range(4):
                        if os.environ.get("KWZ2") == "copy":
                            nc.vector.tensor_copy(zdg[:, qi, :], id32[0:64, :])
                        else:
                            nc.vector.tensor_scalar_mul(
                                zdg[:, qi, :], id32[0:64, :], zinv[:, qi : qi + 1]
                            )

                    if KST < 5:
                        continue
                    # ---- w transpose: pwt[n, j, 32qi+c] = w~[qi, c, n] ----
                    pwt = pwtpool.tile([P, 2, P], F32, tag="pwt")
                    for qi in range(4):
                        pr, side = qi // 2, qi % 2
                        rows = slice(32 * side, 32 * side + 32)
                        for j, (noff, nsz) in enumerate(NCH):
                            nc.tensor.matmul(
                                pwt[0:nsz, j, 32 * qi : 32 * qi + 32],
                                lhsT=wsb[rows, pr, 196 * side + noff : 196 * side + noff + nsz],
                                rhs=(ident_h[rows, 0:32] if os.environ.get("KWRHS") == "ident" else zdg[rows, qi, :]),
                                start=True,
                                stop=True,
                                tile_position=(32 * side, 0),
                            )
                    wt_sb = wtspool.tile([P, 2, P], FP16, tag="wt")
                    nc.scalar.activation(wt_sb[:, 0, :], pwt[:, 0, :], AF.Copy)
                    nc.scalar.activation(wt_sb[0:68, 1, :], pwt[0:68, 1, :], AF.Copy)

                    if KST < 6:
                        continue
                    # ---- ctx: pcx[32qi+h, c] = sum_n w~ * visual ----
                    pcx = pcxpool.tile([P, C], F32, tag="pcx")
                    for qi in range(4):
                        for j, (noff, nsz) in enumerate(NCH):
                            nc.tensor.matmul(
                                pcx[32 * qi : 32 * qi + 32, :],
                                lhsT=wt_sb[0:nsz, j, 32 * qi : 32 * qi + 32],
                                rhs=vb[0:nsz, qi, j, :],
                                start=(j == 0),
                                stop=(j == 1),
                                tile_position=(0, 32 * qi),
                            )
                    ctxe = cepool.tile([P, C], FP16, tag="ctxe")
                    nc.vector.tensor_copy(ctxe[:], pcx[:])

                    if KST < 7:
                        continue
                    # ---- selector transpose -> CTXT[c, csub, 4f+h] ----
                    pct = pctpool.tile([P, CS, 16], F32, tag="pct")
                    for k in range(CS):
                        nc.tensor.matmul(
                            pct[:, k, :],
                            lhsT=ctxe[:, k * P : (k + 1) * P],
                            rhs=sel16[:],
                            start=(k == 0),
                            stop=(k == CS - 1),
                            skip_group_check=True,
                        )
                    nc.vector.tensor_copy(CTXT[:, :, 16 * q : 16 * q + 16], pct[:])

            # ---------------- phase 3: tail ----------------
            with tc.tile_pool(name="ph3", bufs=2) as ph3, tc.tile_pool(
                name="ph3p", bufs=2, space="PSUM"
            ) as ph3p, tc.tile_pool(
                name="ph3pn", bufs=1, space="PSUM"
            ) as ph3pn, tc.tile_pool(name="ph3po", bufs=1, space="PSUM") as ph3po:
                ap_sb = ph3.tile([P, H, F], FP16, tag="apre")
                for h in range(H):
                    pa = ph3p.tile([P, F], F32, tag="pt3")
                    ctxh = CTXT.rearrange("p s (f h) -> p s f h", h=H)
                    for k in range(CS):
                        nc.tensor.matmul(
                            pa[:],
                            lhsT=w_v[:, k, h * P : (h + 1) * P],
                            rhs=ctxh[:, k, :, h],
                            start=(k == 0),
                            stop=(k == CS - 1),
                        )
                    nc.scalar.activation(ap_sb[:, h, :], pa[:], AF.Copy)
                attnT = ph3.tile([P, CS, F], F32, tag="attnT")
                attnT16 = ph3.tile([P, CS, F], FP16, tag="attnT16")
                for ic in range(CS):
                    pb = ph3p.tile([P, F], F32, tag="pt3")
                    for s in range(CS):
                        nc.tensor.matmul(
                            pb[:],
                            lhsT=w_o[:, s, ic * P : (ic + 1) * P],
                            rhs=ap_sb[:, s, :],
                            start=(s == 0),
                            stop=(s == CS - 1),
                        )
                    nc.scalar.activation(
                        attnT[:, ic, :], pb[:], AF.Identity, bias=b_o[:, ic : ic + 1]
                    )
                    nc.vector.tensor_copy(attnT16[:, ic, :], attnT[:, ic, :])
                h1T = ph3.tile([P, CS, F], FP16, tag="h1T")
                for ic in range(CS):
                    pc = ph3p.tile([P, F], F32, tag="pt3")
                    for s in range(CS):
                        nc.tensor.matmul(
                            pc[:],
                            lhsT=w_1[:, s, ic * P : (ic + 1) * P],
                            rhs=attnT16[:, s, :],
                            start=(s == 0),
                            stop=(s == CS - 1),
                        )
                    nc.scalar.activation(
                        h1T[:, ic, :], pc[:], AF.Relu, bias=b_1[:, ic : ic + 1]
                    )
                xT = ph3.tile([P, CS, F], F32, tag="xT")
                for ic in range(CS):
                    pd = ph3p.tile([P, F], F32, tag="pt3")
                    for s in range(CS):
                        nc.tensor.matmul(
                            pd[:],
                            lhsT=w_2[:, s, ic * P : (ic + 1) * P],
                            rhs=h1T[:, s, :],
                            start=(s == 0),
                            stop=(s == CS - 1),
                        )
                    srcb = ph3.tile([P, F], F32, tag="srcb")
                    nc.scalar.activation(
                        srcb[:], pd[:], AF.Identity, bias=b_2[:, ic : ic + 1]
                    )
                    nc.vector.tensor_add(
                        out=xT[:, ic, :], in0=srcb[:], in1=attnT[:, ic, :]
                    )
                # LayerNorm over c (partition+subtile dim) via ones-matmul
                x2 = ph3.tile([P, CS, F], F32, tag="x2")
                nc.vector.tensor_mul(out=x2[:], in0=xT[:], in1=xT[:])
                psums = ph3pn.tile([1, 2, F], F32, tag="psums")
                ps1 = psums[:, 0, :]
                ps2 = psums[:, 1, :]
                for k in range(CS):
                    nc.tensor.matmul(
                        ps1, lhsT=ones[:, 0:1], rhs=xT[:, k, :],
                        start=(k == 0), stop=(k == CS - 1),
                    )
                for k in range(CS):
                    nc.tensor.matmul(
                        ps2, lhsT=ones[:, 0:1], rhs=x2[:, k, :],
                        start=(k == 0), stop=(k == CS - 1),
                    )
                mu = ph3.tile([1, F], F32, tag="mu")
                nc.scalar.activation(mu[:], ps1, AF.Copy, scale=1.0 / C)
                ms = ph3.tile([1, F], F32, tag="ms")
                nc.scalar.activation(ms[:], ps2, AF.Copy, scale=1.0 / C)
                mu2 = ph3.tile([1, F], F32, tag="mu2")
                nc.vector.tensor_mul(out=mu2[:], in0=mu[:], in1=mu[:])
                var = ph3.tile([1, F], F32, tag="var")
                nc.vector.tensor_tensor(
                    var[:], ms[:], mu2[:], mybir.AluOpType.subtract
                )
                epst = ph3.tile([1, 1], F32, tag="epst")
                nc.vector.memset(epst[:], EPS)
                std = ph3.tile([1, F], F32, tag="std")
                nc.scalar.activation(std[:], var[:], AF.Sqrt, bias=epst[0:1, 0:1])
                rstd = ph3.tile([1, F], F32, tag="rstd")
                nc.vector.reciprocal(rstd[:], std[:])
                mrs = ph3.tile([1, F], F32, tag="mrs")
                nc.vector.tensor_mul(out=mrs[:], in0=mu[:], in1=rstd[:])
                rstd_bc = ph3pn.tile([P, F], F32, tag="pbn1")
                nc.tensor.matmul(
                    rstd_bc[:], lhsT=ones_r[0:1, :], rhs=rstd[:], start=True, stop=True
                )
                mrs_bc = ph3pn.tile([P, F], F32, tag="pbn2")
                nc.tensor.matmul(
                    mrs_bc[:], lhsT=ones_r[0:1, :], rhs=mrs[:], start=True, stop=True
                )
                xn = ph3.tile([P, CS, F], F32, tag="xn")
                for k in range(CS):
                    tta = ph3.tile([P, F], F32, tag="tta")
                    nc.vector.tensor_mul(
                        out=tta[:], in0=xT[:, k, :], in1=rstd_bc[:]
                    )
                    ttb = ph3.tile([P, F], F32, tag="ttb")
                    nc.vector.tensor_tensor(
                        ttb[:], tta[:], mrs_bc[:], mybir.AluOpType.subtract,
                    )
                    nc.vector.tensor_scalar(
                        xn[:, k, :], ttb[:],
                        ln_g[:, k : k + 1], lnb_t[:, k : k + 1],
                        mybir.AluOpType.mult, mybir.AluOpType.add,
                    )
                # transpose back to [f, c] and store
                pout = ph3po.tile([F, C], F32, tag="pout")
                for k in range(CS):
                    nc.tensor.transpose(
                        pout[:, k * P : (k + 1) * P], xn[:, k, :], ident[0:P, 0:P]
                    )
                out_sb = ph3.tile([F, C], F32, tag="outsb")
                nc.scalar.activation(out_sb[:], pout[:], AF.Copy)
                nc.sync.dma_start(out[:], out_sb[:])
    _split_multi_waits(nc)
    return nc


def _split_multi_waits(nc):
    """This walrus build allows only one sync-wait per instruction struct;
    move extra waits onto single-wait NoOps on the same engine, inserted
    immediately before the instruction (same-engine program order makes
    this equivalent)."""
    import bass_rust

    n = [0]
    for func in nc.m.functions:
        for blk in func.blocks:
            insts = blk.instructions
            out = []
            for inst in insts:
                si = inst.sync_info
                waits = list(si.on_wait) if si and si.on_wait else []
                if len(waits) > 1 and inst.engine != mybir.EngineType.Unassigned:
                    for w in waits[:-1]:
                        nop = mybir.InstNoOp(
                            name=f"I-waitsplit-{n[0]}", ins=[], outs=[]
                        )
                        n[0] += 1
                        nop.engine = inst.engine
                        nop.sync_info = bass_rust.SyncInfo(
                            on_wait=[w], on_update=[]
                        )
                        nc.register_instruction(nop)
                        out.append(nop)
                    si.on_wait = [waits[-1]]
                out.append(inst)
            if len(out) != len(insts):
                insts.clear()
                insts.extend(out)
    return nc


def _patch_tile_drain():
    """This walrus build rejects >1 sync-wait on CTRL-class (Drain) instrs;
    split the Tile kernel-tail drain's waits into a chain of 1-wait drains."""
    import bass_rust
    from concourse.tile import ScopedClock

    if getattr(tile.TileContext, "_drain_patched", False):
        return

    def patched(self, tick_clock, wait_clock):
        drain_inst = self.nc.sync.drain()
        wait_clock.add_sem_waits(
            drain_inst.ins, ScopedClock({None: tick_clock.global_clock})
        )
        si = drain_inst.ins.sync_info
        waits = list(si.on_wait) if si and si.on_wait else []
        if len(waits) > 1:
            si.on_wait = [waits[0]]
            for w in waits[1:]:
                d2 = self.nc.sync.drain()
                d2.ins.sync_info = bass_rust.SyncInfo(on_wait=[w], on_update=[])
        self.nc.all_engine_barrier()
        popped = self.nc._tile_sem_poison_stack.pop()
        assert popped is self._sem_poison
        self.nc.clear_and_free_semaphores(list(self.sems.allocated().values()))
        self.nc.all_engine_barrier()

    tile.TileContext._drain_and_barrier = patched
    tile.TileContext._drain_patched = True


_patch_tile_drain()


def host_weights(in_proj_w, in_proj_b, out_proj_w, out_proj_b, lin1_w, lin1_b,
                 lin2_w, lin2_b, ln_g, ln_b):
    """Pre-lay weights into the [128, sub, free] SBUF layouts the kernel uses."""
    scale = 1.0 / np.sqrt(np.float32(D))
    Wq, Wk, Wv = (np.asarray(in_proj_w[i * C : (i + 1) * C]) for i in range(3))
    bq = np.asarray(in_proj_b[0:C])
    bv = np.asarray(in_proj_b[2 * C : 3 * C])

    def t_psf(w):  # [C_out rows, x] -> [p, sub, x] with rows = sub*128+p
        return np.ascontiguousarray(w.reshape(CS, P, -1).transpose(1, 0, 2))

    wqts = t_psf(np.ascontiguousarray(Wq.T) * scale)      # [c] rows -> q cols
    wkdhc = t_psf(Wk)                                     # [(h d), c] -> [d, h, c]
    wvt = t_psf(np.ascontiguousarray(Wv.T))               # [c, (h d)]
    woutt = t_psf(np.ascontiguousarray(np.asarray(out_proj_w).T))
    lin1t = t_psf(np.ascontiguousarray(np.asarray(lin1_w).T))
    lin2t = t_psf(np.ascontiguousarray(np.asarray(lin2_w).T))

    def b_ps(b):  # [512] -> [128, 4] with c = sub*128+p
        return np.ascontiguousarray(np.asarray(b).reshape(CS, P).T)

    b_o_eff = np.asarray(out_proj_b) + np.asarray(out_proj_w) @ bv
    biases = np.concatenate(
        [b_ps(bq * scale), b_ps(b_o_eff), b_ps(np.asarray(lin1_b)),
         b_ps(np.asarray(lin2_b)), b_ps(np.asarray(ln_g)),
         b_ps(np.asarray(ln_b))], axis=1,
    ).astype(np.float32)
    import ml_dtypes
    hdt = (ml_dtypes.bfloat16 if os.environ.get("KHALF", "fp16") == "bf16"
           else np.float16)
    return dict(
        wqts=wqts.astype(np.float32), wkdhc=wkdhc.astype(np.float32),
        wvt=wvt.astype(hdt), woutt=woutt.astype(hdt),
        lin1t=lin1t.astype(hdt), lin2t=lin2t.astype(hdt),
        biases=biases,
    )


_NC_CACHE = {}


def kernel(audio_top_k, visual_patch_feat, in_proj_w, in_proj_b, out_proj_w,
           out_proj_b, lin1_w, lin1_b, lin2_w, lin2_b, ln_g, ln_b):
    from concourse.bass_utils import run_bass_kernel_spmd

    wmap = host_weights(in_proj_w, in_proj_b, out_proj_w, out_proj_b,
                        lin1_w, lin1_b, lin2_w, lin2_b, ln_g, ln_b)
    audio = np.asarray(audio_top_k, np.float32)
    visual = np.asarray(visual_patch_feat, np.float32)
    bpc = B // NCORES
    in_maps = []
    for c in range(NCORES):
        sl = slice(c * bpc, (c + 1) * bpc)
        in_maps.append(
            dict(
                audio=np.ascontiguousarray(audio[sl].reshape(FRAMES, C)),
                visual=np.ascontiguousarray(visual[sl].reshape(FRAMES, N, C)),
                **wmap,
            )
        )
    if "nc" not in _NC_CACHE:
        _NC_CACHE["nc"] = build_nc()
    res = run_bass_kernel_spmd(_NC_CACHE["nc"], in_maps, list(range(NCORES)))
    outs = [res.results[c]["out"].reshape(bpc, T, C) for c in range(NCORES)]
    return np.concatenate(outs, axis=0)
